# revision 37
# baseline (speedup 1.0000x reference)
"""Trainium2 Bass kernel for an Adapter block (LN -> 768x64 -> ReLU -> 64x768).

Data-parallel over batch (8 entries -> 8 cores). Per core x is [4096, 768].

Design (all-bf16 compute; fp8 variants measured slower — the DVE loses its
2x perf mode on 1-byte operands, starving the PE and resetting its p-state):
  - Host ships x group-contiguous bf16 [NG, 128, 6, 512] (feature f =
    c*128 + p); one 6KB/partition in-DMA per group on the sync-engine
    HWDGE; output leaves as two 3KB/partition out-DMAs per group on the
    gpsimd SWDGE (separate queue so x descriptors never queue behind
    them), except the last two slices on sync for a faster drain.
  - Down-proj weight-stationary: lhsT = [gamma*W_d | ones] (M=65), rhs =
    x chunks -> psum rows 0:64 = raw down d, row 64 = S1 = sum_f x.
  - S2 = sum_f x^2: DVE squares x (bf16, 2x mode), 6 ones-matmuls with a
    65-column lhsT so they share the down matmuls' PE tile config.
  - LN corrections as rank-1 matmuls accumulated into psum:
      zcorr: psum_d[0:64] += (-sg/768) (x) S1   => z = d - mu*sg
      vcorr: psum_s2     += (-1/768)  (x) S1^2  => V = 768*var
    with S1, S1^2 staged as [1, gt] bf16 rows (both on ACT).
  - rstd = Rsqrt(V/768 + eps) on ACT (raw InstActivation; table accuracy
    ~1e-3 vs the 2e-2 budget). y = z*rstd (DVE); lup = Relu(y+c) (ACT).
  - Up-proj: lhsT = W_u[:, m*128:(m+1)*128] (K=64), 6 matmuls; psum
    drains via 3 ACT + 3 DVE copies into a group-contiguous osb tile
    (GPSIMD cannot read PSUM).
  - PE p-state: the governor reaches full 2.4GHz only after ~3us of
    sustained MATMUL activity (ldweights spins do not count, idle gaps
    reset it), so the kernel warms up with 70 dummy matmuls into a
    scratch psum tile, timed to end as the first x tile lands.
  - Token slices 256,256,512x6,256,256: halves at the edges shorten
    pipeline fill and drain; the last slice's rstd/y/relu run one
    iteration early to compress the tail.

Steady state is PE-bound at ~4.9us per 512 tokens (downs 6x213ns + S2
6x213 + corr 2x330 + ups 6x216 + stop/config overheads); DMA ~4.6us.
Measured ~63-64us end-to-end (v1 baseline 66.5us; 96.9us original).
"""

import numpy as np

D_MODEL = 768
BOTTLENECK = 64
LN_EPS = 1e-5
SCALE = 1.0
N_CORES = 8
TOK = 4096
P = 128
NCH = D_MODEL // P   # 6 feature chunks
GT = 512             # tokens per group (DRAM layout granularity)
NG = TOK // GT       # 8 groups
K = BOTTLENECK

# token slices: half-groups at the edges for shorter fill/drain
SL = [(0, 256), (256, 256)] + [(g * GT, GT) for g in range(1, NG - 1)] \
     + [(TOK - GT, 256), (TOK - 256, 256)]
NS = len(SL)

_CACHE = {}


def _build(bup_zero):
    import concourse.bacc as bacc
    import concourse.bass as bass
    import concourse.tile as tile
    from concourse import mybir
    from contextlib import ExitStack

    f32 = mybir.dt.float32
    bf16 = mybir.dt.bfloat16
    AF = mybir.ActivationFunctionType
    OP = mybir.AluOpType

    INV_D = 1.0 / D_MODEL

    nc = bacc.Bacc("TRN2", target_bir_lowering=False, debug=False,
                   num_devices=N_CORES)

    def act_raw(out, in_, func, bias, scale):
        eng = nc.scalar
        inputs = [eng.lower_ap(in_)]
        for arg in (bias, scale, 0.0):
            if isinstance(arg, bass.AP):
                inputs.append(eng.lower_ap(arg))
            else:
                inputs.append(mybir.ImmediateValue(dtype=mybir.dt.float32,
                                                   value=float(arg)))
        return eng.add_instruction(mybir.InstActivation(
            name=eng.bass.get_next_instruction_name(),
            func=func, ins=inputs, outs=[eng.lower_ap(out)]))

    x_d = nc.dram_tensor("x", [NG, P, NCH, GT], bf16,
                         kind="ExternalInput").ap()
    wga_d = nc.dram_tensor("wga", [P, NCH, K + 1], bf16,
                           kind="ExternalInput").ap()
    wu_d = nc.dram_tensor("wu", [K, D_MODEL], bf16, kind="ExternalInput").ap()
    sc_d = nc.dram_tensor("sc", [K, 2], f32, kind="ExternalInput").ap()
    ng_d = nc.dram_tensor("ng", [1, 2 * K], bf16, kind="ExternalInput").ap()
    if not bup_zero:
        bup_d = nc.dram_tensor("bup", [P, NCH], f32, kind="ExternalInput").ap()
    out_d = nc.dram_tensor("out", [NG, P, NCH, GT], bf16,
                           kind="ExternalOutput").ap()

    with tile.TileContext(nc, pool_alloc_mode="queue") as tc, ExitStack() as ctx:
        consts = ctx.enter_context(tc.tile_pool(name="consts", bufs=1))
        xt_pool = ctx.enter_context(tc.tile_pool(name="xt", bufs=4))
        sq_pool = ctx.enter_context(tc.tile_pool(name="sq", bufs=2))
        s1_pool = ctx.enter_context(tc.tile_pool(name="s1t1", bufs=2))
        fix_pool = ctx.enter_context(tc.tile_pool(name="fix", bufs=2))
        lup_pool = ctx.enter_context(tc.tile_pool(name="lup", bufs=2))
        out_pool = ctx.enter_context(tc.tile_pool(name="outp", bufs=3))
        ps_d = ctx.enter_context(tc.tile_pool(name="ps_d", bufs=2, space="PSUM"))
        ps_s2 = ctx.enter_context(tc.tile_pool(name="ps_s2", bufs=1, space="PSUM"))
        ps_up = ctx.enter_context(tc.tile_pool(name="ps_up", bufs=5, space="PSUM"))

        # ---- constants (SWDGE on the idle gpsimd path; x loads own sync) ----
        wga_sb = consts.tile([P, NCH, K + 1], bf16)
        nc.gpsimd.dma_start(out=wga_sb, in_=wga_d)
        ng_sb = consts.tile([1, 2 * K], bf16)   # [-sg/768 | -1/768]
        nc.gpsimd.dma_start(out=ng_sb, in_=ng_d)
        sc_sb = consts.tile([K, 2], f32)
        nc.gpsimd.dma_start(out=sc_sb, in_=sc_d)
        wu_sb = consts.tile([K, D_MODEL], bf16)
        nc.gpsimd.dma_start(out=wu_sb, in_=wu_d)
        # 65 columns so S2 matmuls share the down matmuls' PE tile config
        ones_sb = consts.tile([P, K + 1], bf16)
        nc.vector.memset(ones_sb, 1.0)
        eps_t = consts.tile([K, 1], f32)
        nc.vector.memset(eps_t, LN_EPS)
        if not bup_zero:
            bup_sb = consts.tile([P, NCH], f32)
            nc.gpsimd.dma_start(out=bup_sb, in_=bup_d)
        scr_t = consts.tile([K, 1], f32)
        act_raw(out=scr_t, in_=eps_t, func=AF.Rsqrt, bias=0.0, scale=1.0)

        st = {}

        def dma_in(i):
            t0, gt = SL[i]
            g, r = divmod(t0, GT)
            xa = xt_pool.tile([P, NCH, GT], bf16)
            nc.sync.dma_start(out=xa[:, :, 0:gt], in_=x_d[g, :, :, r:r + gt])
            st[("x", i)] = xa

        def front_sq(i):
            gt = SL[i][1]
            xa = st[("x", i)]
            H = NCH // 2
            sq = sq_pool.tile([P, NCH, GT], bf16)
            nc.vector.tensor_tensor(out=sq[:, 0:H, 0:gt], in0=xa[:, 0:H, 0:gt],
                                    in1=xa[:, 0:H, 0:gt], op=OP.mult)
            nc.vector.tensor_tensor(out=sq[:, H:NCH, 0:gt],
                                    in0=xa[:, H:NCH, 0:gt],
                                    in1=xa[:, H:NCH, 0:gt], op=OP.mult)
            st[("sq", i)] = sq

        def front_down(i):
            gt = SL[i][1]
            xa = st.pop(("x", i))
            dps = ps_d.tile([P, GT], f32)
            for c in range(NCH):
                # stop stays False: zcorr is the true closer of this group,
                # and stop-marked matmuls cost ~+95ns of PE spacing
                nc.tensor.matmul(dps[0:K + 1, 0:gt], lhsT=wga_sb[:, c, :],
                                 rhs=xa[:, c, 0:gt],
                                 start=(c == 0), stop=False,
                                 skip_group_check=True)
            st[("d", i)] = dps

        def stage_s1t1(i):
            gt = SL[i][1]
            dps = st[("d", i)]
            s1t1 = s1_pool.tile([1, 2, GT], bf16)
            nc.scalar.activation(out=s1t1[:, 1, 0:gt], in_=dps[K:K + 1, 0:gt],
                                 func=AF.Square, scale=1.0)
            nc.scalar.activation(out=s1t1[:, 0, 0:gt], in_=dps[K:K + 1, 0:gt],
                                 func=AF.Copy, bias=0.0, scale=1.0)
            st[("s1t1", i)] = s1t1

        def front_s2(i):
            gt = SL[i][1]
            sq = st.pop(("sq", i))
            s2ps = ps_s2.tile([K + 1, GT], f32, tag="s2")
            for c in range(NCH):
                nc.tensor.matmul(s2ps[:, 0:gt], lhsT=ones_sb,
                                 rhs=sq[:, c, 0:gt],
                                 start=(c == 0), stop=False,
                                 skip_group_check=True)
            st[("s2", i)] = s2ps

        def front_vcorr(i):
            gt = SL[i][1]
            s2ps = st[("s2", i)]
            s1t1 = st[("s1t1", i)]
            nc.tensor.matmul(s2ps[0:K, 0:gt], lhsT=ng_sb[:, K:2 * K],
                             rhs=s1t1[:, 1, 0:gt],
                             start=False, stop=True,
                             skip_group_check=True)

        def front_zcorr(i):
            gt = SL[i][1]
            dps = st[("d", i)]
            s1t1 = st.pop(("s1t1", i))
            nc.tensor.matmul(dps[0:K, 0:gt], lhsT=ng_sb[:, 0:K],
                             rhs=s1t1[:, 0, 0:gt],
                             start=False, stop=True,
                             skip_group_check=True)

        def mid_rstd(j):
            gt = SL[j][1]
            s2ps = st.pop(("s2", j))
            rstd = fix_pool.tile([K, GT], f32, tag="rstd")
            act_raw(out=rstd[:, 0:gt], in_=s2ps[0:K, 0:gt], func=AF.Rsqrt,
                    bias=eps_t, scale=INV_D)
            st[("rstd", j)] = rstd

        def mid_y(j):
            gt = SL[j][1]
            dps = st.pop(("d", j))
            rstd = st.pop(("rstd", j))
            y = fix_pool.tile([K, GT], f32, tag="y")
            nc.vector.tensor_tensor(out=y[:, 0:gt], in0=dps[0:K, 0:gt],
                                    in1=rstd[:, 0:gt], op=OP.mult)
            st[("y", j)] = y

        def mid_relu(j):
            gt = SL[j][1]
            y = st.pop(("y", j))
            lup = lup_pool.tile([K, GT], bf16)
            nc.scalar.activation(out=lup[:, 0:gt], in_=y[:, 0:gt],
                                 func=AF.Relu, bias=sc_sb[:, 1:2], scale=1.0)
            st[("lup", j)] = lup

        def back_up(k):
            gt = SL[k][1]
            lup = st.pop(("lup", k))
            ups = []
            for m in range(NCH):
                upt = ps_up.tile([P, GT], f32, tag="u")
                nc.tensor.matmul(upt[:, 0:gt],
                                 lhsT=wu_sb[:, m * P:(m + 1) * P],
                                 rhs=lup[:, 0:gt], start=True, stop=True)
                ups.append(upt)
            st[("ups", k)] = ups
            osb = out_pool.tile([P, NCH, GT], bf16)
            st[("osb", k)] = osb

        def back_copy(k, m, eng):
            gt = SL[k][1]
            src = st[("ups", k)][m][:, 0:gt]
            dst = st[("osb", k)][:, m, 0:gt]
            if eng == "act":
                if bup_zero:
                    nc.scalar.activation(out=dst, in_=src, func=AF.Copy,
                                         bias=0.0, scale=SCALE)
                else:
                    nc.scalar.activation(out=dst, in_=src, func=AF.Identity,
                                         bias=bup_sb[:, m:m + 1], scale=SCALE)
            else:
                if bup_zero:
                    nc.vector.tensor_copy(out=dst, in_=src)
                else:
                    nc.vector.tensor_scalar(out=dst, in0=src,
                                            scalar1=bup_sb[:, m:m + 1],
                                            scalar2=None, op0=OP.add)

        def back_out(k, lo, hi):
            t0, gt = SL[k]
            g, r = divmod(t0, GT)
            osb = st[("osb", k)]
            eng = nc.sync if k >= NS - 2 else nc.gpsimd
            eng.dma_start(out=out_d[g, :, lo:hi, r:r + gt],
                          in_=osb[:, lo:hi, 0:gt])

        for i in range(3):
            dma_in(i)
        # Warm the PE with real (dummy) matmuls until the first x tile lands:
        # the p-state governor ramps on sustained MATMUL activity only
        # (ldweights spins don't count), reaching full clock after ~3us.
        wps = ps_s2.tile([K + 1, GT], f32, tag="s2")
        for _ in range(70):
            nc.tensor.matmul(wps[:, 0:K], lhsT=ones_sb, rhs=ones_sb[:, 0:K],
                             start=True, stop=True)
        for i in range(NS + 2):
            f = i < NS
            j = i - 1
            k = i - 2
            bk = 0 <= k < NS
            md = 0 <= j < NS - 1   # mid for the last slice runs early, below
            if i + 3 < NS:
                dma_in(i + 3)
            if f:
                front_sq(i)
            if bk:
                back_up(k)
                back_copy(k, 0, "act")
                back_copy(k, 1, "act")
            if md:
                mid_rstd(j)
            if bk:
                back_copy(k, 2, "dve")
                back_out(k, 0, 3)
            if f:
                front_down(i)
            if md:
                mid_y(j)
                mid_relu(j)
            if f:
                stage_s1t1(i)
                front_s2(i)
            if bk:
                back_copy(k, 3, "act")
                back_copy(k, 4, "dve")
                back_copy(k, 5, "dve")
            if f:
                front_vcorr(i)
                front_zcorr(i)
                if i == NS - 1:   # pull the last slice's mid into this iter
                    mid_rstd(i)
                    mid_y(i)
                    mid_relu(i)
            if bk:
                back_out(k, 3, 6)
                st.pop(("ups", k))
                st.pop(("osb", k))

    nc.compile()
    return nc


def _get_nc(bup_zero):
    key = ("nc", bup_zero)
    if key not in _CACHE:
        _CACHE[key] = _build(bup_zero)
    return _CACHE[key]


def _in_maps(x, ln_gamma, ln_beta, w_down, b_down, w_up, b_up):
    import ml_dtypes
    f = np.float32
    bf = ml_dtypes.bfloat16
    x = np.asarray(x, dtype=f)
    ln_gamma = np.asarray(ln_gamma, dtype=f)
    ln_beta = np.asarray(ln_beta, dtype=f)
    w_down = np.asarray(w_down, dtype=f)
    b_down = np.asarray(b_down, dtype=f)
    w_up = np.asarray(w_up, dtype=f)
    b_up = np.asarray(b_up, dtype=f)

    wg = (ln_gamma[:, None] * w_down).astype(bf)         # [768, 64] on-device
    wga = np.ones((D_MODEL, K + 1), f)
    wga[:, 0:K] = wg.astype(f)
    wga = wga.reshape(NCH, P, K + 1).transpose(1, 0, 2)  # [p, c, 65]
    sg = wg.astype(f).sum(axis=0)                        # [64] matches bf16 wg
    cc = ln_beta @ w_down + b_down                       # [64]
    sc = np.stack([np.zeros_like(sg), cc], axis=1)       # col0 unused
    ng = np.concatenate([-sg / D_MODEL,
                         np.full((K,), -1.0 / D_MODEL, f)])[None, :]
    bup_zero = not np.any(b_up)

    common = {
        "wga": np.ascontiguousarray(wga.astype(bf)),
        "wu": np.ascontiguousarray(w_up.astype(bf)),
        "sc": np.ascontiguousarray(sc.astype(f)),
        "ng": np.ascontiguousarray(ng.astype(bf)),
    }
    if not bup_zero:
        common["bup"] = np.ascontiguousarray(
            b_up.reshape(NCH, P).T.astype(f))             # [p, c]
    maps = []
    for i in range(N_CORES):
        xg = x[i].reshape(NG, GT, NCH, P).transpose(0, 3, 2, 1)  # [g,p,c,t]
        maps.append(dict(common, x=xg.astype(bf)))
    return bup_zero, maps


def run(trace=False, **inputs):
    """Run the SPMD kernel; returns (output, BassKernelResults)."""
    from concourse.bass_utils import run_bass_kernel_spmd
    bup_zero, in_maps = _in_maps(**inputs)
    nc = _get_nc(bup_zero)
    res = run_bass_kernel_spmd(nc, in_maps, core_ids=list(range(N_CORES)),
                               trace=trace)
    outs = []
    for i in range(N_CORES):
        o = np.asarray(res.results[i]["out"])            # [g, p, c, t] bf16
        outs.append(o.transpose(0, 3, 2, 1).reshape(TOK, D_MODEL))
    return np.stack(outs, axis=0).astype(np.float32), res


def kernel(**inputs) -> np.ndarray:
    out, _ = run(trace=False, **inputs)
    return out
